# revision 28
# baseline (speedup 1.0000x reference)
"""Causal self-attention (B=2, L=2048, E=2048, H=16, HD=128) on 8 trn2 cores.

Sharding: core c = (b, g) with b = c // 4 (batch), g = c % 4 (head group of 4).
Each core computes QKV projection for its 4 heads on its batch, causal
attention with RoPE, and a partial output projection (its heads' slice of
w_proj rows). Host sums the 4 partial projections per batch.

All matmuls run in bf16 with fp32 PSUM accumulation (measured end-to-end
rel. error ~5e-3 vs the fp32 reference).

Key device-side structure (per core):
  - phase 1, per 512-wide l-chunk: q/k/v projections as K-accumulated
    matmuls; rope fused right behind each q/k chunk:
        rot = (q * cs) - swap(q * ss)     [2 DVE muls + DMA half-swap + sub]
    with cs/ss host-prebuilt [128, L] tables (softmax scale folded in).
    The rope pair-partner half-swap runs as two SBUF->SBUF DMAs (partition
    halves exchanged) instead of a PE permutation matmul.
  - phase 2: scores computed transposed (sT[j,i] = k_j . q_i) so P@V needs
    no transpose; softmax without max-subtraction (|s| <= ~10); denominator
    via all-ones matmul which also broadcasts Z across partitions; causal
    masking by skipping upper-triangle blocks + 4 static diagonal masks;
    software-pipelined with a 3-deep score-matmul lookahead.
  - phase 3: partial out-projection, [f, l] layout, fp16 partials, emitted
    per l-chunk as soon as its 4 heads finish (interleaved into attention).

All HBM-side tensors are host-retiled so that every DMA transfer is fully
contiguous in HBM (the strided-row layouts otherwise cap each DMA queue at
~25-40 GB/s; contiguous tiles run at the per-descriptor floor ~95+ GB/s).

Device layouts (per core):
  xt    [NLC, NE, 128, 512] bf16  x[b].T tiles (contiguous per tile)
  wqkv  [NE, 128, 1536]     bf16  per e-chunk: q-heads 0..3 | k-heads 0..3
                                  (head rows perm'd to (even|odd) order,
                                  transposed) | v-heads 0..3 (natural)
  wout  [128, HPG*E]        bf16  w_proj[:, g*512:(g+1)*512].T, SBUF layout
  cs,ss [128, L]            bf16  rope cos / (-sin|+sin) tables * 128**-0.25
  masks [128, 4*512]        bf16  causal diagonal-block masks
  ones  [128, 128]          bf16  all-ones (softmax denominator broadcast-sum)
Output:
  out   [NE, NLC, 128, 512] fp16  (partial projection tiles, transposed;
                                  host adds + reassembles in fp32)
"""

from contextlib import ExitStack

import numpy as np
import ml_dtypes

import concourse.bass as bass
import concourse.mybir as mybir
import concourse.tile as tile
from concourse import bacc
from concourse.bass_utils import run_bass_kernel_spmd

BF16 = ml_dtypes.bfloat16
B, L, E, H, HD = 2, 2048, 2048, 16, 128
G = 4            # head groups (cores per batch)
HPG = H // G     # heads per group = 4
NCORES = 8
NE = E // 128    # 16 e-chunks
NLC = L // 512   # 4 l-chunks of 512
NLT = L // 128   # 16 l-tiles of 128
SCALE = float(128.0 ** -0.25)   # per-operand score scale (q and k each)

FP32 = mybir.dt.float32
BF = mybir.dt.bfloat16


def build_nc():
    nc = bacc.Bacc(
        "TRN2",
        target_bir_lowering=False,
        debug=False,
        enable_asserts=False,
        num_devices=NCORES,
    )
    d = {}
    d["xt"] = nc.dram_tensor("xt", [NLC, NE, 128, 512], BF,
                             kind="ExternalInput").ap()
    d["wqv"] = nc.dram_tensor("wqv", [NE, 128, 2 * HPG * 128], BF,
                              kind="ExternalInput").ap()
    d["wk"] = nc.dram_tensor("wk", [NE, 128, HPG * 128], BF,
                             kind="ExternalInput").ap()
    d["wout"] = nc.dram_tensor("wout", [128, HPG * E], BF,
                               kind="ExternalInput").ap()
    d["cs"] = nc.dram_tensor("cs", [128, L], BF, kind="ExternalInput").ap()
    d["ss"] = nc.dram_tensor("ss", [128, L], BF, kind="ExternalInput").ap()
    d["masks"] = nc.dram_tensor("masks", [128, 4 * 512], BF,
                                kind="ExternalInput").ap()
    d["ones"] = nc.dram_tensor("ones", [128, 128], BF,
                               kind="ExternalInput").ap()
    d["out"] = nc.dram_tensor("out", [NE // 2, NLC, 128, 2, 512],
                              mybir.dt.float16, kind="ExternalOutput").ap()

    with tile.TileContext(nc) as tc:
        build_kernel(tc, d)
    nc.compile()
    return nc


def build_kernel(tc, d):
    nc = tc.nc
    EXP = mybir.ActivationFunctionType.Exp

    with ExitStack() as ctx:
        const = ctx.enter_context(tc.tile_pool(name="const", bufs=1))
        qkres = ctx.enter_context(tc.tile_pool(name="qkres", bufs=1))
        vres = ctx.enter_context(tc.tile_pool(name="vres", bufs=1))
        yres = ctx.enter_context(tc.tile_pool(name="yres", bufs=1))

        # ---- constants / weights ----
        # every DMA below reads a fully-contiguous HBM region (host retiled)
        # weight chunk DMAs are emitted interleaved with lc0's xt tiles across
        # all three DMA queues (sync/gpsimd/scalar). lc0 runs q+v passes
        # first (they share xt and the wqv chunks); the k weights stream in
        # behind and are consumed by lc0's trailing k-pass. This keeps the
        # startup demand under the achievable aggregate DMA rate.
        wqv_sb = const.tile([128, NE, 2 * HPG * 128], BF, name="wqv_sb",
                            tag="wqv_sb")
        wk_sb = const.tile([128, NE, HPG * 128], BF, name="wk_sb",
                           tag="wk_sb")
        # cs/ss and masks/ones/wout tiles are allocated here but their DMAs
        # are emitted inside the lc loop (behind lc0 / lc1 xt tiles on the
        # gpsimd queue) so they don't delay the startup-critical loads.
        cs_sb = const.tile([128, L], BF, name="cs_sb", tag="cs_sb")
        ss_sb = const.tile([128, L], BF, name="ss_sb", tag="ss_sb")
        masks_sb = const.tile([128, 4, 512], BF, name="masks_sb",
                              tag="masks_sb")
        ones_sb = const.tile([128, 128], BF, name="ones_sb", tag="ones_sb")
        wout_sb = const.tile([128, HPG, E], BF, name="wout_sb", tag="wout_sb")

        # ---- PE warmup ----
        # ~64 tiny dependency-free matmuls on scratch data fill the initial
        # DMA-wait window with PE activity, so the HAM clock gate is already
        # at 8/8 (2.4 GHz) when the first real matmul issues.
        warm_sb = const.tile([128, 64], BF, name="warm_sb", tag="warm_sb")
        nc.vector.memset(warm_sb, 0.0)

        # ---- residents ----
        q_sb = [qkres.tile([128, L], BF, name=f"q_sb{h}", tag=f"q_sb{h}")
                for h in range(HPG)]
        k_sb = [qkres.tile([128, L], BF, name=f"k_sb{h}", tag=f"k_sb{h}")
                for h in range(HPG)]
        v_sb = vres.tile([128, NLT, HPG * 128], BF, name="v_sb", tag="v_sb")
        y_sb = [yres.tile([128, L], BF, name=f"y_sb{h}", tag=f"y_sb{h}")
                for h in range(HPG)]

        # ================= phase 1: QKV projection + fused rope ============
        # xs/atile/aswp live only in phase 1; closing them frees their SBUF
        # for the phase-2 pools below (stack allocation).
        with tc.tile_pool(name="psum1", bufs=1, space="PSUM") as ps1, \
                tc.tile_pool(name="xs", bufs=32) as xs, \
                tc.tile_pool(name="atile", bufs=3) as atile, \
                tc.tile_pool(name="aswp", bufs=2) as aswp:

            def acc_tile(nm):
                return ps1.tile([128, 512], FP32, name=nm, tag="pacc", bufs=8)

            warm_ps = acc_tile("warm_ps")
            for i in range(64):
                nc.tensor.matmul(warm_ps[0:64, 0:64], lhsT=warm_sb,
                                 rhs=warm_sb, start=True, stop=True)

            QS = (nc.sync, nc.gpsimd, nc.scalar)

            def load_xt(lc):
                xt_t = []
                for e in range(NE):
                    t = xs.tile([128, 512], BF, name=f"xt_{lc}_{e}", tag="xt")
                    if lc == 0:
                        QS[e % 3].dma_start(out=t, in_=d["xt"][lc, e])
                        QS[(e + 1) % 3].dma_start(out=wqv_sb[:, e, :],
                                                  in_=d["wqv"][e])
                    else:
                        eng = nc.sync if e % 2 == 0 else nc.gpsimd
                        eng.dma_start(out=t, in_=d["xt"][lc, e])
                    xt_t.append(t)
                if lc == 0:
                    nc.gpsimd.dma_start(out=cs_sb, in_=d["cs"])
                    nc.gpsimd.dma_start(out=ss_sb, in_=d["ss"])
                elif lc == 1:
                    # wk is consumed only by the deferred k-passes; stream it
                    # behind lc1's x tiles
                    for e in range(NE):
                        QS[(2 * e) % 3].dma_start(out=wk_sb[:, e, :],
                                                  in_=d["wk"][e])
                    nc.gpsimd.dma_start(
                        out=masks_sb,
                        in_=d["masks"].rearrange("p (r f) -> p r f", r=4))
                    nc.gpsimd.dma_start(out=ones_sb, in_=d["ones"])
                    nc.gpsimd.dma_start(
                        out=wout_sb,
                        in_=d["wout"].rearrange("p (h f) -> p h f", h=HPG))
                return xt_t

            def qv_pass(lc, xt_t):
                # q-projection and v-pass interleaved per e-chunk: both
                # consume only xt + wqv, maximizing compute per delivered
                # byte while the input stream is still catching up
                accq = [acc_tile(f"p0_{lc}_{h}") for h in range(HPG)]
                accv = [acc_tile(f"pv_{lc * 4 + ls}") for ls in range(4)]
                for e in range(NE):
                    for h in range(HPG):
                        nc.tensor.matmul(
                            accq[h],
                            lhsT=wqv_sb[:, e, h * 128:(h + 1) * 128],
                            rhs=xt_t[e],
                            start=(e == 0), stop=(e == NE - 1))
                    for ls in range(4):
                        nc.tensor.matmul(
                            accv[ls],
                            lhsT=xt_t[e][:, ls * 128:(ls + 1) * 128],
                            rhs=wqv_sb[:, e, 512:1024],
                            start=(e == 0), stop=(e == NE - 1))
                return accq, accv

            def k_pass(lc, xt_t):
                acc = [acc_tile(f"p1_{lc}_{h}") for h in range(HPG)]
                for e in range(NE):
                    for h in range(HPG):
                        nc.tensor.matmul(
                            acc[h],
                            lhsT=wk_sb[:, e, h * 128:(h + 1) * 128],
                            rhs=xt_t[e],
                            start=(e == 0), stop=(e == NE - 1))
                return acc

            def v_store(lc, accv):
                for ls in range(4):
                    nc.scalar.copy(out=v_sb[:, lc * 4 + ls, :], in_=accv[ls])

            def rope_a(lc, acc, which):
                # a[:, h, :] = q*ss (bf16), dst-slice = q*cs; releases acc
                ls_lo = lc * 512
                a_t = atile.tile([128, HPG, 512], BF,
                                 name=f"a_{which}_{lc}", tag="a")
                for h in range(HPG):
                    nc.vector.tensor_mul(out=a_t[:, h, :], in0=acc[h],
                                         in1=ss_sb[:, ls_lo:ls_lo + 512])
                    dst = (q_sb if which == "q" else k_sb)[h]
                    nc.vector.tensor_mul(
                        out=dst[:, ls_lo:ls_lo + 512], in0=acc[h],
                        in1=cs_sb[:, ls_lo:ls_lo + 512])
                return a_t

            def rope_b(lc, a_t, which):
                # dst -= half_swap(a): partner lives 64 partitions away
                # (even|odd weight-row order); swap all 4 heads' a-tiles
                # with 2 batched SBUF->SBUF DMAs
                ls_lo = lc * 512
                a2 = aswp.tile([128, HPG, 512], BF,
                               name=f"a2_{which}_{lc}", tag="a2")
                nc.scalar.dma_start(out=a2[0:64], in_=a_t[64:128])
                nc.scalar.dma_start(out=a2[64:128], in_=a_t[0:64])
                for h in range(HPG):
                    dst = (q_sb if which == "q" else k_sb)[h]
                    sl = dst[:, ls_lo:ls_lo + 512]
                    nc.vector.tensor_sub(out=sl, in0=sl, in1=a2[:, h, :])

            # schedule: qv0 qv1 [k0] qv2 [k1] qv3 [k2] [k3] — each lc's
            # k-pass is deferred until after the NEXT lc's qv-pass, so the
            # k weights and later x tiles never gate the PE stream.
            xt_all = {}
            aq_t = {}
            xt_all[0] = load_xt(0)
            accq, accv = qv_pass(0, xt_all[0])
            aq_t[0] = rope_a(0, accq, "q")
            v_store(0, accv)
            for lc in range(1, NLC):
                xt_all[lc] = load_xt(lc)
                accq, accv = qv_pass(lc, xt_all[lc])
                aq_t[lc] = rope_a(lc, accq, "q")
                v_store(lc, accv)
                acck = k_pass(lc - 1, xt_all.pop(lc - 1))
                ak = rope_a(lc - 1, acck, "k")
                rope_b(lc - 1, aq_t.pop(lc - 1), "q")
                rope_b(lc - 1, ak, "k")
            acck = k_pass(NLC - 1, xt_all.pop(NLC - 1))
            ak = rope_a(NLC - 1, acck, "k")
            rope_b(NLC - 1, aq_t.pop(NLC - 1), "q")
            rope_b(NLC - 1, ak, "k")

        # ======== phase 2+3: causal attention with interleaved projection ==
        # jobs are ic-major: once all 4 heads finished l-chunk ic, that
        # chunk's output projection is emitted immediately — it fills
        # attention pipeline bubbles and spreads the output DMA.
        with tc.tile_pool(name="psum2", bufs=1, space="PSUM") as ps2, \
                tc.tile_pool(name="pexp", bufs=10) as pexp, \
                tc.tile_pool(name="zpool", bufs=3) as zpool, \
                tc.tile_pool(name="outst", bufs=2) as outst:
            jobs = [(h, ic) for ic in range(NLC) for h in range(HPG)]
            steps = [(ji, jb)
                     for ji, (_h, ic) in enumerate(jobs)
                     for jb in range(4 * ic + 4)]
            LA = 3
            pss_map = {}
            zy = {}

            def emit_s(ji, jb):
                h, ic = jobs[ji]
                # diagonal blocks (r >= 1) have no valid columns below
                # f = 128*r: compute only the valid column range
                r = jb - 4 * ic
                lo = r * 128 if r > 0 else 0
                t = ps2.tile([128, 512], FP32, name=f"pss_{ji}_{jb}",
                             tag="pss", bufs=4)
                nc.tensor.matmul(
                    t[:, lo:],
                    lhsT=k_sb[h][:, jb * 128:(jb + 1) * 128],
                    rhs=q_sb[h][:, ic * 512 + lo:(ic + 1) * 512],
                    start=True, stop=True)
                pss_map[(ji, jb)] = t

            ptr = 0
            for idx, (ji, jb) in enumerate(steps):
                while ptr < len(steps) and ptr <= idx + LA:
                    emit_s(*steps[ptr])
                    ptr += 1
                h, ic = jobs[ji]
                njb = 4 * ic + 4
                if jb == 0:
                    zy[ji] = (
                        ps2.tile([128, 512], FP32, name=f"psz_{ji}",
                                 tag="pzy", bufs=4),
                        ps2.tile([128, 512], FP32, name=f"psy_{ji}",
                                 tag="pzy", bufs=4),
                    )
                psz, psy = zy[ji]
                pss = pss_map.pop((ji, jb))
                r = jb - 4 * ic
                lo = r * 128 if r > 0 else 0
                pt = pexp.tile([128, 512], BF, name=f"pt_{ji}_{jb}", tag="pexp")
                nc.scalar.activation(out=pt[:, lo:], in_=pss[:, lo:], func=EXP)
                if r >= 0:
                    # diagonal block: only the first 128 columns of the valid
                    # range hold the per-element triangle; the rest are all-1
                    nc.vector.tensor_mul(
                        out=pt[:, lo:lo + 128], in0=pt[:, lo:lo + 128],
                        in1=masks_sb[:, r, lo:lo + 128])
                nc.tensor.matmul(psz[:, lo:], lhsT=ones_sb, rhs=pt[:, lo:],
                                 start=(jb == 0), stop=(jb == njb - 1))
                nc.tensor.matmul(psy[:, lo:],
                                 lhsT=v_sb[:, jb, h * 128:(h + 1) * 128],
                                 rhs=pt[:, lo:],
                                 start=(jb == 0), stop=(jb == njb - 1))
                if jb == njb - 1:
                    zv = zpool.tile([128, 512], FP32, name=f"zinv_{ji}",
                                    tag="zinv")
                    nc.vector.reciprocal_approx_fast(out=zv, in_=psz)
                    nc.vector.tensor_mul(
                        out=y_sb[h][:, ic * 512:(ic + 1) * 512],
                        in0=psy, in1=zv)
                    if h == HPG - 1:
                        # all heads done for this l-chunk: emit its projection.
                        # ft-pairs share one staging tile so each out DMA is a
                        # 256KB contiguous store (halves descriptor overhead).
                        lc = ic
                        ot = None
                        for ft in range(NE):
                            po = ps2.tile([128, 512], FP32,
                                          name=f"po_{ft}_{lc}", tag="pzy",
                                          bufs=4)
                            for hh in range(HPG):
                                nc.tensor.matmul(
                                    po,
                                    lhsT=wout_sb[:, hh,
                                                 ft * 128:(ft + 1) * 128],
                                    rhs=y_sb[hh][:, lc * 512:(lc + 1) * 512],
                                    start=(hh == 0), stop=(hh == HPG - 1))
                            if ft % 2 == 0:
                                ot = outst.tile([128, 2, 512],
                                                mybir.dt.float16,
                                                name=f"ot_{ft // 2}_{lc}",
                                                tag="ot", bufs=4)
                                nc.vector.tensor_copy(out=ot[:, 0, :], in_=po)
                            else:
                                nc.scalar.copy(out=ot[:, 1, :], in_=po)
                                eng = (nc.sync, nc.gpsimd,
                                       nc.scalar)[(ft // 2) % 3]
                                eng.dma_start(out=d["out"][ft // 2, lc],
                                              in_=ot)


# ------------------------------------------------------------------ host side

_PERM_IDX = np.concatenate([np.arange(0, 128, 2), np.arange(1, 128, 2)])


def prep_in_maps(x, rope, w_attn, w_proj):
    x = np.asarray(x, np.float32)
    rope = np.asarray(rope, np.float32)
    w_attn = np.asarray(w_attn, np.float32)
    w_proj = np.asarray(w_proj, np.float32)

    sin = rope[:, :, 0]                      # [L, 64]
    cos = rope[:, :, 1]
    cs = (np.concatenate([cos.T, cos.T], 0) * SCALE).astype(BF16)   # [128, L]
    ss = (np.concatenate([-sin.T, sin.T], 0) * SCALE).astype(BF16)

    p = np.arange(128)[:, None]
    f = np.arange(512)[None, :]
    masks = np.zeros((128, 4, 512), np.float32)
    for r in range(4):
        masks[:, r, :] = (r * 128 + p <= f).astype(np.float32)
    masks = masks.reshape(128, 4 * 512).astype(BF16)

    ones = np.ones((128, 128), np.float32).astype(BF16)

    # xt tiles: [NLC, NE, 128, 512], each tile contiguous
    xt_b = []
    for b in range(B):
        xT = np.ascontiguousarray(x[b].T)                  # [E, L]
        t = xT.reshape(NE, 128, NLC, 512).transpose(2, 0, 1, 3)
        xt_b.append(np.ascontiguousarray(t).astype(BF16))

    wqv_g, wk_g, wout_g = {}, {}, {}
    for g in range(G):
        heads = [g * HPG + hl for hl in range(HPG)]
        wq = [np.ascontiguousarray(
                 w_attn[h * 128:(h + 1) * 128, :][_PERM_IDX, :].T) for h in heads]
        wk = [np.ascontiguousarray(
                 w_attn[E + h * 128:E + (h + 1) * 128, :][_PERM_IDX, :].T)
              for h in heads]
        wv = [w_attn[2 * E + h * 128:2 * E + (h + 1) * 128, :].T for h in heads]
        wqv = np.concatenate(wq + wv, axis=1)              # [E, 1024]
        wqv_g[g] = np.ascontiguousarray(
            wqv.reshape(NE, 128, 2 * HPG * 128)).astype(BF16)
        wkc = np.concatenate(wk, axis=1)                   # [E, 512]
        wk_g[g] = np.ascontiguousarray(
            wkc.reshape(NE, 128, HPG * 128)).astype(BF16)
        wo = np.ascontiguousarray(
            w_proj[:, g * 512:(g + 1) * 512].T)            # [512, E]
        wo = wo.reshape(HPG, 128, E).transpose(1, 0, 2)    # [128, HPG, E]
        wout_g[g] = np.ascontiguousarray(wo.reshape(128, HPG * E)).astype(BF16)

    in_maps = []
    for c in range(NCORES):
        b, g = divmod(c, G)
        in_maps.append({
            "xt": xt_b[b],
            "wqv": wqv_g[g],
            "wk": wk_g[g],
            "wout": wout_g[g],
            "cs": cs,
            "ss": ss,
            "masks": masks,
            "ones": ones,
        })
    return in_maps


def assemble_output(results):
    out = np.zeros((B, L, E), np.float32)
    for c in range(NCORES):
        b, g = divmod(c, G)
        r = np.asarray(results[c]["out"], np.float32)  # [NE/2, NLC, 128, 2, 512]
        full = r.transpose(0, 3, 2, 1, 4).reshape(E, L)
        out[b] += full.T
    return out


_NC = None


def get_nc():
    global _NC
    if _NC is None:
        _NC = build_nc()
    return _NC


def run(x, rope, w_attn, w_proj, trace=False, tmpdir=None):
    nc = get_nc()
    in_maps = prep_in_maps(x, rope, w_attn, w_proj)
    kwargs = {}
    if trace:
        import sys
        import types
        from concourse import bass_utils as _bu
        try:
            from trn_agent_boot.trn_boot import _ntff_profile_via_ctypes
            hook = _ntff_profile_via_ctypes("/opt/axon/libaxon_pjrt.so")
            mod = types.ModuleType("antenv.axon_hooks")
            mod.get_axon_ntff_profile_hook = lambda: hook
            sys.modules["antenv.axon_hooks"] = mod
            _bu.upload_artifacts = lambda dd: dd
        except Exception as e:  # pragma: no cover
            print("trace hook unavailable:", e)
        kwargs = dict(trace=True, tmpdir=tmpdir)
    res = run_bass_kernel_spmd(nc, in_maps, core_ids=list(range(NCORES)), **kwargs)
    return assemble_output(res.results), res


def kernel(x, rope, w_attn, w_proj):
    out, _ = run(x, rope, w_attn, w_proj, trace=False)
    return out


# revision 29
# speedup vs baseline: 1.0232x; 1.0232x over previous
"""Causal self-attention (B=2, L=2048, E=2048, H=16, HD=128) on 8 trn2 cores.

Sharding: core c = (b, g) with b = c // 4 (batch), g = c % 4 (head group of 4).
Each core computes QKV projection for its 4 heads on its batch, causal
attention with RoPE, and a partial output projection (its heads' slice of
w_proj rows). Host sums the 4 partial projections per batch.

All matmuls run in bf16 with fp32 PSUM accumulation (measured end-to-end
rel. error ~5e-3 vs the fp32 reference).

Key device-side structure (per core):
  - phase 1, per 512-wide l-chunk: q/k/v projections as K-accumulated
    matmuls; rope fused right behind each q/k chunk:
        rot = (q * cs) - swap(q * ss)     [2 DVE muls + DMA half-swap + sub]
    with cs/ss host-prebuilt [128, L] tables (softmax scale folded in).
    The rope pair-partner half-swap runs as two SBUF->SBUF DMAs (partition
    halves exchanged) instead of a PE permutation matmul.
  - phase 2: scores computed transposed (sT[j,i] = k_j . q_i) so P@V needs
    no transpose; softmax without max-subtraction (|s| <= ~10); denominator
    via all-ones matmul which also broadcasts Z across partitions; causal
    masking by skipping upper-triangle blocks + 4 static diagonal masks;
    software-pipelined with a 3-deep score-matmul lookahead.
  - phase 3: partial out-projection, [f, l] layout, fp16 partials, emitted
    per l-chunk as soon as its 4 heads finish (interleaved into attention).

All HBM-side tensors are host-retiled so that every DMA transfer is fully
contiguous in HBM (the strided-row layouts otherwise cap each DMA queue at
~25-40 GB/s; contiguous tiles run at the per-descriptor floor ~95+ GB/s).

Device layouts (per core):
  xt    [NLC, NE, 128, 512] bf16  x[b].T tiles (contiguous per tile)
  wqkv  [NE, 128, 1536]     bf16  per e-chunk: q-heads 0..3 | k-heads 0..3
                                  (head rows perm'd to (even|odd) order,
                                  transposed) | v-heads 0..3 (natural)
  wout  [128, HPG*E]        bf16  w_proj[:, g*512:(g+1)*512].T, SBUF layout
  cs,ss [128, L]            bf16  rope cos / (-sin|+sin) tables * 128**-0.25
  masks [128, 4*512]        bf16  causal diagonal-block masks
  ones  [128, 128]          bf16  all-ones (softmax denominator broadcast-sum)
Output:
  out   [NE, NLC, 128, 512] fp16  (partial projection tiles, transposed;
                                  host adds + reassembles in fp32)
"""

from contextlib import ExitStack

import numpy as np
import ml_dtypes

import concourse.bass as bass
import concourse.mybir as mybir
import concourse.tile as tile
from concourse import bacc
from concourse.bass_utils import run_bass_kernel_spmd

BF16 = ml_dtypes.bfloat16
B, L, E, H, HD = 2, 2048, 2048, 16, 128
G = 4            # head groups (cores per batch)
HPG = H // G     # heads per group = 4
NCORES = 8
NE = E // 128    # 16 e-chunks
NLC = L // 512   # 4 l-chunks of 512
NLT = L // 128   # 16 l-tiles of 128
SCALE = float(128.0 ** -0.25)   # per-operand score scale (q and k each)

FP32 = mybir.dt.float32
BF = mybir.dt.bfloat16


def build_nc():
    nc = bacc.Bacc(
        "TRN2",
        target_bir_lowering=False,
        debug=False,
        enable_asserts=False,
        num_devices=NCORES,
    )
    d = {}
    d["xt"] = nc.dram_tensor("xt", [NLC, NE, 128, 512], BF,
                             kind="ExternalInput").ap()
    d["wqv"] = nc.dram_tensor("wqv", [NE, 128, 2 * HPG * 128], BF,
                              kind="ExternalInput").ap()
    d["wk"] = nc.dram_tensor("wk", [NE, 128, HPG * 128], BF,
                             kind="ExternalInput").ap()
    d["wout"] = nc.dram_tensor("wout", [128, HPG * E], BF,
                               kind="ExternalInput").ap()
    d["cs"] = nc.dram_tensor("cs", [128, L], BF, kind="ExternalInput").ap()
    d["ss"] = nc.dram_tensor("ss", [128, L], BF, kind="ExternalInput").ap()
    d["masks"] = nc.dram_tensor("masks", [128, 4 * 512], BF,
                                kind="ExternalInput").ap()
    d["ones"] = nc.dram_tensor("ones", [128, 128], BF,
                               kind="ExternalInput").ap()
    d["out"] = nc.dram_tensor("out", [NE // 2, NLC, 128, 2, 512],
                              mybir.dt.float16, kind="ExternalOutput").ap()

    with tile.TileContext(nc) as tc:
        build_kernel(tc, d)
    nc.compile()
    return nc


def build_kernel(tc, d):
    nc = tc.nc
    EXP = mybir.ActivationFunctionType.Exp

    with ExitStack() as ctx:
        const = ctx.enter_context(tc.tile_pool(name="const", bufs=1))
        qkres = ctx.enter_context(tc.tile_pool(name="qkres", bufs=1))
        vres = ctx.enter_context(tc.tile_pool(name="vres", bufs=1))
        yres = ctx.enter_context(tc.tile_pool(name="yres", bufs=1))

        # ---- constants / weights ----
        # every DMA below reads a fully-contiguous HBM region (host retiled)
        # weight chunk DMAs are emitted interleaved with lc0's xt tiles across
        # all three DMA queues (sync/gpsimd/scalar). lc0 runs q+v passes
        # first (they share xt and the wqv chunks); the k weights stream in
        # behind and are consumed by lc0's trailing k-pass. This keeps the
        # startup demand under the achievable aggregate DMA rate.
        wqv_sb = const.tile([128, NE, 2 * HPG * 128], BF, name="wqv_sb",
                            tag="wqv_sb")
        wk_sb = const.tile([128, NE, HPG * 128], BF, name="wk_sb",
                           tag="wk_sb")
        # cs/ss and masks/ones/wout tiles are allocated here but their DMAs
        # are emitted inside the lc loop (behind lc0 / lc1 xt tiles on the
        # gpsimd queue) so they don't delay the startup-critical loads.
        cs_sb = const.tile([128, L], BF, name="cs_sb", tag="cs_sb")
        ss_sb = const.tile([128, L], BF, name="ss_sb", tag="ss_sb")
        masks_sb = const.tile([128, 4, 512], BF, name="masks_sb",
                              tag="masks_sb")
        ones_sb = const.tile([128, 128], BF, name="ones_sb", tag="ones_sb")
        wout_sb = const.tile([128, HPG, E], BF, name="wout_sb", tag="wout_sb")

        # ---- PE warmup ----
        # ~64 tiny dependency-free matmuls on scratch data fill the initial
        # DMA-wait window with PE activity, so the HAM clock gate is already
        # at 8/8 (2.4 GHz) when the first real matmul issues.
        warm_sb = const.tile([128, 64], BF, name="warm_sb", tag="warm_sb")
        nc.vector.memset(warm_sb, 0.0)

        # ---- residents ----
        q_sb = [qkres.tile([128, L], BF, name=f"q_sb{h}", tag=f"q_sb{h}")
                for h in range(HPG)]
        k_sb = [qkres.tile([128, L], BF, name=f"k_sb{h}", tag=f"k_sb{h}")
                for h in range(HPG)]
        v_sb = vres.tile([128, NLT, HPG * 128], BF, name="v_sb", tag="v_sb")
        y_sb = [yres.tile([128, L], BF, name=f"y_sb{h}", tag=f"y_sb{h}")
                for h in range(HPG)]

        # ================= phase 1: QKV projection + fused rope ============
        # xs/atile/aswp live only in phase 1; closing them frees their SBUF
        # for the phase-2 pools below (stack allocation).
        with tc.tile_pool(name="psum1", bufs=1, space="PSUM") as ps1, \
                tc.tile_pool(name="atile", bufs=3) as atile, \
                tc.tile_pool(name="aswp", bufs=2) as aswp, \
                tc.tile_pool(name="xs", bufs=32) as xs:

            def acc_tile(nm):
                return ps1.tile([128, 512], FP32, name=nm, tag="pacc", bufs=8)

            warm_ps = acc_tile("warm_ps")
            for i in range(64):
                nc.tensor.matmul(warm_ps[0:64, 0:64], lhsT=warm_sb,
                                 rhs=warm_sb, start=True, stop=True)

            QS = (nc.sync, nc.gpsimd, nc.scalar)

            def load_xt(lc):
                # gpsimd is SWDGE: it concatenates contiguous descriptors
                # into 4KB packets (~2x the HWDGE per-queue rate), so it
                # carries the bulk weight streams; xt tiles alternate on the
                # two HWDGE queues (sync/scalar).
                xt_t = []
                for e in range(NE):
                    t = xs.tile([128, 512], BF, name=f"xt_{lc}_{e}", tag="xt")
                    eng = nc.sync if e % 2 == 0 else nc.scalar
                    eng.dma_start(out=t, in_=d["xt"][lc, e])
                    if lc == 0:
                        weng = (nc.gpsimd if e % 3 != 2 else
                                (nc.sync if e % 6 == 2 else nc.scalar))
                        weng.dma_start(out=wqv_sb[:, e, :], in_=d["wqv"][e])
                    xt_t.append(t)
                if lc == 0:
                    nc.gpsimd.dma_start(out=cs_sb, in_=d["cs"])
                    nc.gpsimd.dma_start(out=ss_sb, in_=d["ss"])
                elif lc == 1:
                    # wk is consumed only by the deferred k-passes; stream it
                    # behind lc1's x tiles on the fast queue
                    for e in range(NE):
                        nc.gpsimd.dma_start(out=wk_sb[:, e, :], in_=d["wk"][e])
                    nc.gpsimd.dma_start(
                        out=masks_sb,
                        in_=d["masks"].rearrange("p (r f) -> p r f", r=4))
                    nc.gpsimd.dma_start(out=ones_sb, in_=d["ones"])
                    nc.gpsimd.dma_start(
                        out=wout_sb,
                        in_=d["wout"].rearrange("p (h f) -> p h f", h=HPG))
                return xt_t

            def qv_pass(lc, xt_t):
                # q-projection and v-pass interleaved per e-chunk: both
                # consume only xt + wqv, maximizing compute per delivered
                # byte while the input stream is still catching up
                accq = [acc_tile(f"p0_{lc}_{h}") for h in range(HPG)]
                accv = [acc_tile(f"pv_{lc * 4 + ls}") for ls in range(4)]
                for e in range(NE):
                    for h in range(HPG):
                        nc.tensor.matmul(
                            accq[h],
                            lhsT=wqv_sb[:, e, h * 128:(h + 1) * 128],
                            rhs=xt_t[e],
                            start=(e == 0), stop=(e == NE - 1))
                    for ls in range(4):
                        nc.tensor.matmul(
                            accv[ls],
                            lhsT=xt_t[e][:, ls * 128:(ls + 1) * 128],
                            rhs=wqv_sb[:, e, 512:1024],
                            start=(e == 0), stop=(e == NE - 1))
                return accq, accv

            def k_pass(lc, xt_t):
                acc = [acc_tile(f"p1_{lc}_{h}") for h in range(HPG)]
                for e in range(NE):
                    for h in range(HPG):
                        nc.tensor.matmul(
                            acc[h],
                            lhsT=wk_sb[:, e, h * 128:(h + 1) * 128],
                            rhs=xt_t[e],
                            start=(e == 0), stop=(e == NE - 1))
                return acc

            def v_store(lc, accv):
                for ls in range(4):
                    nc.scalar.copy(out=v_sb[:, lc * 4 + ls, :], in_=accv[ls])

            def rope_a(lc, acc, which, rev=False):
                # a[:, h, :] = q*ss (bf16), dst-slice = q*cs; releases acc
                ls_lo = lc * 512
                a_t = atile.tile([128, HPG, 512], BF,
                                 name=f"a_{which}_{lc}", tag="a")
                for h in (reversed(range(HPG)) if rev else range(HPG)):
                    nc.vector.tensor_mul(out=a_t[:, h, :], in0=acc[h],
                                         in1=ss_sb[:, ls_lo:ls_lo + 512])
                    dst = (q_sb if which == "q" else k_sb)[h]
                    nc.vector.tensor_mul(
                        out=dst[:, ls_lo:ls_lo + 512], in0=acc[h],
                        in1=cs_sb[:, ls_lo:ls_lo + 512])
                return a_t

            def rope_b(lc, a_t, which):
                # dst -= half_swap(a): partner lives 64 partitions away
                # (even|odd weight-row order); swap all 4 heads' a-tiles
                # with 2 batched SBUF->SBUF DMAs
                ls_lo = lc * 512
                a2 = aswp.tile([128, HPG, 512], BF,
                               name=f"a2_{which}_{lc}", tag="a2")
                nc.scalar.dma_start(out=a2[0:64], in_=a_t[64:128])
                nc.scalar.dma_start(out=a2[64:128], in_=a_t[0:64])
                for h in range(HPG):
                    dst = (q_sb if which == "q" else k_sb)[h]
                    sl = dst[:, ls_lo:ls_lo + 512]
                    nc.vector.tensor_sub(out=sl, in0=sl, in1=a2[:, h, :])

            # schedule: qv0 qv1 [k0] qv2 [k1] qv3 [k2] [k3] — each lc's
            # k-pass is deferred until after the NEXT lc's qv-pass, so the
            # k weights and later x tiles never gate the PE stream.
            xt_all = {}
            aq_t = {}
            xt_all[0] = load_xt(0)
            accq, accv = qv_pass(0, xt_all[0])
            aq_t[0] = rope_a(0, accq, "q")
            v_store(0, accv)
            for lc in range(1, NLC):
                xt_all[lc] = load_xt(lc)
                accq, accv = qv_pass(lc, xt_all[lc])
                aq_t[lc] = rope_a(lc, accq, "q")
                v_store(lc, accv)
                acck = k_pass(lc - 1, xt_all.pop(lc - 1))
                ak = rope_a(lc - 1, acck, "k")
                rope_b(lc - 1, aq_t.pop(lc - 1), "q")
                rope_b(lc - 1, ak, "k")
            acck = k_pass(NLC - 1, xt_all.pop(NLC - 1))
            ak = rope_a(NLC - 1, acck, "k", rev=True)
            rope_b(NLC - 1, aq_t.pop(NLC - 1), "q")
            rope_b(NLC - 1, ak, "k")

        # ======== phase 2+3: causal attention with interleaved projection ==
        # jobs are ic-major: once all 4 heads finished l-chunk ic, that
        # chunk's output projection is emitted immediately — it fills
        # attention pipeline bubbles and spreads the output DMA.
        with tc.tile_pool(name="psum2", bufs=1, space="PSUM") as ps2, \
                tc.tile_pool(name="pexp", bufs=10) as pexp, \
                tc.tile_pool(name="zpool", bufs=3) as zpool, \
                tc.tile_pool(name="outst", bufs=2) as outst:
            jobs = [(h, ic) for ic in range(NLC) for h in range(HPG)]
            steps = [(ji, jb)
                     for ji, (_h, ic) in enumerate(jobs)
                     for jb in range(4 * ic + 4)]
            LA = 3
            pss_map = {}
            zy = {}

            def emit_s(ji, jb):
                h, ic = jobs[ji]
                # diagonal blocks (r >= 1) have no valid columns below
                # f = 128*r: compute only the valid column range
                r = jb - 4 * ic
                lo = r * 128 if r > 0 else 0
                t = ps2.tile([128, 512], FP32, name=f"pss_{ji}_{jb}",
                             tag="pss", bufs=4)
                nc.tensor.matmul(
                    t[:, lo:],
                    lhsT=k_sb[h][:, jb * 128:(jb + 1) * 128],
                    rhs=q_sb[h][:, ic * 512 + lo:(ic + 1) * 512],
                    start=True, stop=True)
                pss_map[(ji, jb)] = t

            ptr = 0
            for idx, (ji, jb) in enumerate(steps):
                while ptr < len(steps) and ptr <= idx + LA:
                    emit_s(*steps[ptr])
                    ptr += 1
                h, ic = jobs[ji]
                njb = 4 * ic + 4
                if jb == 0:
                    zy[ji] = (
                        ps2.tile([128, 512], FP32, name=f"psz_{ji}",
                                 tag="pzy", bufs=4),
                        ps2.tile([128, 512], FP32, name=f"psy_{ji}",
                                 tag="pzy", bufs=4),
                    )
                psz, psy = zy[ji]
                pss = pss_map.pop((ji, jb))
                r = jb - 4 * ic
                lo = r * 128 if r > 0 else 0
                pt = pexp.tile([128, 512], BF, name=f"pt_{ji}_{jb}", tag="pexp")
                nc.scalar.activation(out=pt[:, lo:], in_=pss[:, lo:], func=EXP)
                if r >= 0:
                    # diagonal block: only the first 128 columns of the valid
                    # range hold the per-element triangle; the rest are all-1
                    nc.vector.tensor_mul(
                        out=pt[:, lo:lo + 128], in0=pt[:, lo:lo + 128],
                        in1=masks_sb[:, r, lo:lo + 128])
                nc.tensor.matmul(psz[:, lo:], lhsT=ones_sb, rhs=pt[:, lo:],
                                 start=(jb == 0), stop=(jb == njb - 1))
                nc.tensor.matmul(psy[:, lo:],
                                 lhsT=v_sb[:, jb, h * 128:(h + 1) * 128],
                                 rhs=pt[:, lo:],
                                 start=(jb == 0), stop=(jb == njb - 1))
                if jb == njb - 1:
                    zv = zpool.tile([128, 512], FP32, name=f"zinv_{ji}",
                                    tag="zinv")
                    nc.vector.reciprocal_approx_fast(out=zv, in_=psz)
                    nc.vector.tensor_mul(
                        out=y_sb[h][:, ic * 512:(ic + 1) * 512],
                        in0=psy, in1=zv)
                    if h == HPG - 1:
                        # all heads done for this l-chunk: emit its projection.
                        # ft-pairs share one staging tile so each out DMA is a
                        # 256KB contiguous store (halves descriptor overhead).
                        lc = ic
                        ot = None
                        for ft in range(NE):
                            po = ps2.tile([128, 512], FP32,
                                          name=f"po_{ft}_{lc}", tag="pzy",
                                          bufs=4)
                            for hh in range(HPG):
                                nc.tensor.matmul(
                                    po,
                                    lhsT=wout_sb[:, hh,
                                                 ft * 128:(ft + 1) * 128],
                                    rhs=y_sb[hh][:, lc * 512:(lc + 1) * 512],
                                    start=(hh == 0), stop=(hh == HPG - 1))
                            if ft % 2 == 0:
                                ot = outst.tile([128, 2, 512],
                                                mybir.dt.float16,
                                                name=f"ot_{ft // 2}_{lc}",
                                                tag="ot", bufs=4)
                                nc.vector.tensor_copy(out=ot[:, 0, :], in_=po)
                            else:
                                nc.scalar.copy(out=ot[:, 1, :], in_=po)
                                eng = (nc.sync, nc.gpsimd,
                                       nc.scalar)[(ft // 2) % 3]
                                eng.dma_start(out=d["out"][ft // 2, lc],
                                              in_=ot)


# ------------------------------------------------------------------ host side

_PERM_IDX = np.concatenate([np.arange(0, 128, 2), np.arange(1, 128, 2)])


def prep_in_maps(x, rope, w_attn, w_proj):
    x = np.asarray(x, np.float32)
    rope = np.asarray(rope, np.float32)
    w_attn = np.asarray(w_attn, np.float32)
    w_proj = np.asarray(w_proj, np.float32)

    sin = rope[:, :, 0]                      # [L, 64]
    cos = rope[:, :, 1]
    cs = (np.concatenate([cos.T, cos.T], 0) * SCALE).astype(BF16)   # [128, L]
    ss = (np.concatenate([-sin.T, sin.T], 0) * SCALE).astype(BF16)

    p = np.arange(128)[:, None]
    f = np.arange(512)[None, :]
    masks = np.zeros((128, 4, 512), np.float32)
    for r in range(4):
        masks[:, r, :] = (r * 128 + p <= f).astype(np.float32)
    masks = masks.reshape(128, 4 * 512).astype(BF16)

    ones = np.ones((128, 128), np.float32).astype(BF16)

    # xt tiles: [NLC, NE, 128, 512], each tile contiguous
    xt_b = []
    for b in range(B):
        xT = np.ascontiguousarray(x[b].T)                  # [E, L]
        t = xT.reshape(NE, 128, NLC, 512).transpose(2, 0, 1, 3)
        xt_b.append(np.ascontiguousarray(t).astype(BF16))

    wqv_g, wk_g, wout_g = {}, {}, {}
    for g in range(G):
        heads = [g * HPG + hl for hl in range(HPG)]
        wq = [np.ascontiguousarray(
                 w_attn[h * 128:(h + 1) * 128, :][_PERM_IDX, :].T) for h in heads]
        wk = [np.ascontiguousarray(
                 w_attn[E + h * 128:E + (h + 1) * 128, :][_PERM_IDX, :].T)
              for h in heads]
        wv = [w_attn[2 * E + h * 128:2 * E + (h + 1) * 128, :].T for h in heads]
        wqv = np.concatenate(wq + wv, axis=1)              # [E, 1024]
        wqv_g[g] = np.ascontiguousarray(
            wqv.reshape(NE, 128, 2 * HPG * 128)).astype(BF16)
        wkc = np.concatenate(wk, axis=1)                   # [E, 512]
        wk_g[g] = np.ascontiguousarray(
            wkc.reshape(NE, 128, HPG * 128)).astype(BF16)
        wo = np.ascontiguousarray(
            w_proj[:, g * 512:(g + 1) * 512].T)            # [512, E]
        wo = wo.reshape(HPG, 128, E).transpose(1, 0, 2)    # [128, HPG, E]
        wout_g[g] = np.ascontiguousarray(wo.reshape(128, HPG * E)).astype(BF16)

    in_maps = []
    for c in range(NCORES):
        b, g = divmod(c, G)
        in_maps.append({
            "xt": xt_b[b],
            "wqv": wqv_g[g],
            "wk": wk_g[g],
            "wout": wout_g[g],
            "cs": cs,
            "ss": ss,
            "masks": masks,
            "ones": ones,
        })
    return in_maps


def assemble_output(results):
    out = np.zeros((B, L, E), np.float32)
    for c in range(NCORES):
        b, g = divmod(c, G)
        r = np.asarray(results[c]["out"], np.float32)  # [NE/2, NLC, 128, 2, 512]
        full = r.transpose(0, 3, 2, 1, 4).reshape(E, L)
        out[b] += full.T
    return out


_NC = None


def get_nc():
    global _NC
    if _NC is None:
        _NC = build_nc()
    return _NC


def run(x, rope, w_attn, w_proj, trace=False, tmpdir=None):
    nc = get_nc()
    in_maps = prep_in_maps(x, rope, w_attn, w_proj)
    kwargs = {}
    if trace:
        import sys
        import types
        from concourse import bass_utils as _bu
        try:
            from trn_agent_boot.trn_boot import _ntff_profile_via_ctypes
            hook = _ntff_profile_via_ctypes("/opt/axon/libaxon_pjrt.so")
            mod = types.ModuleType("antenv.axon_hooks")
            mod.get_axon_ntff_profile_hook = lambda: hook
            sys.modules["antenv.axon_hooks"] = mod
            _bu.upload_artifacts = lambda dd: dd
        except Exception as e:  # pragma: no cover
            print("trace hook unavailable:", e)
        kwargs = dict(trace=True, tmpdir=tmpdir)
    res = run_bass_kernel_spmd(nc, in_maps, core_ids=list(range(NCORES)), **kwargs)
    return assemble_output(res.results), res


def kernel(x, rope, w_attn, w_proj):
    out, _ = run(x, rope, w_attn, w_proj, trace=False)
    return out
